# revision 19
# baseline (speedup 1.0000x reference)
"""Distributed Trainium2 kernel for nn_AddAttention_154618823089.

Computation (see reference):
    q = rope(bf16(hidden @ Wq.T)); k = rope(bf16(hidden @ Wk.T))
    o[b,l] = sum_{j<=l} exp(q_l . k_j / sqrt(DIM))          (no softmax norm)
    out = relu(o @ fc1_w.T + fc1_b) @ fc2_w.T + fc2_b

Sharding: every core c handles the strided row set {r : r % 8 == c} of
BOTH batches (512 rows each).  Striding makes the causal workload identical on
every core, and taking rows from both batches makes the k exchange a single
fast 8-rank AllGather per batch (4-rank groups hit the slow fold_n=2 ring).

fp8 pipeline (rel err ~2.8e-3 vs the 2e-2 gate; exp row-sums average the
quantization noise away):
  - h and Wq/Wk enter as fp8e4m3; projections run DoubleRow fp8 matmuls
    (256-deep contraction per instruction, 2x bf16 rate)
  - PSUM f32 -> bf16 staging (gpsimd copies), RoPE in bf16 on vector,
    final rotate writes land directly in fp8 q_rope/k_rope
  - k bounce + AllGather move fp8 (half the bytes of bf16), kt tiles are
    [128, t8, r4, jj512] fp8 so score matmuls slice [:, 2dtp:2dtp+2, ...]
    as the DoubleRow k-pair
  - a dependency-free 512B AllGather issued first on the cc stream pulls
    the ~55us ncfw startup barrier to the front of the NEFF
  - causal score blocks [128q x 512k] as before, exp fused with row-sum
    via accum_out; MLP per subtile, outputs DMAed straight from PSUM
"""

import sys
import types

import numpy as np
from ml_dtypes import bfloat16, float8_e4m3

import concourse.bacc as bacc
import concourse.bass as bass
import concourse.mybir as mybir
import concourse.tile as tile
from concourse.bass_utils import run_bass_kernel_spmd


def _install_ntff_hook():
    """The container's antenv lacks axon_hooks; provide it so trace=True can
    capture NTFF profiles (exec_time_ns) through the axon PJRT library."""
    if "antenv.axon_hooks" in sys.modules:
        return
    try:
        sys.path.insert(0, "/root/.axon_site/trn_agent_boot")
        import trn_boot

        mod = types.ModuleType("antenv.axon_hooks")
        _h = {"hook": None}
        mod.set_axon_ntff_profile_hook = lambda h: _h.__setitem__("hook", h)
        mod.get_axon_ntff_profile_hook = lambda: _h["hook"]
        sys.modules["antenv.axon_hooks"] = mod
        import antenv

        antenv.axon_hooks = mod
        mod.set_axon_ntff_profile_hook(
            trn_boot._ntff_profile_via_ctypes("/opt/axon/libaxon_pjrt.so"))
    except Exception:
        pass


_install_ntff_hook()

B, L, DIM, INNER = 2, 4096, 1024, 16
ROPE_BASE = 32.0
NCORES = 8
RB = L // NCORES       # rows per core per batch (512)
RLOC = 2 * RB          # local q/k rows per core (both batches, 1024)
NSUB = RB // 128       # q subtiles per core per batch (4)
NDT = DIM // 128       # d tiles (8)
NDP = NDT // 2         # DoubleRow d-tile pairs (4)
SCALE = 1.0 / float(np.sqrt(DIM))
MASK_NEG = -1.0e6
CHUNK = 3              # psum banks per score chunk
F32 = mybir.dt.float32
BF16 = mybir.dt.bfloat16
F8 = mybir.dt.float8e4
DR = mybir.MatmulPerfMode.DoubleRow

_NC_CACHE = {}


def _build_nc():
    nc = bacc.Bacc("TRN2", target_bir_lowering=False, debug=False,
                   num_devices=NCORES, num_swdge_queues=4)

    hT = nc.dram_tensor("hT", [DIM, RLOC], F8, kind="ExternalInput")
    wqT = nc.dram_tensor("wqT", [DIM, DIM], F8, kind="ExternalInput")
    wkT = nc.dram_tensor("wkT", [DIM, DIM], F8, kind="ExternalInput")
    cosh = nc.dram_tensor("cosh", [DIM // 2, RLOC], BF16, kind="ExternalInput")
    sinh = nc.dram_tensor("sinh", [DIM // 2, RLOC], BF16, kind="ExternalInput")
    mask0 = nc.dram_tensor("mask0", [128, 512], F32, kind="ExternalInput")
    mask1 = nc.dram_tensor("mask1", [128, 512], F32, kind="ExternalInput")
    w1b_d = nc.dram_tensor("w1b", [128, 32], F32, kind="ExternalInput")
    b1b_d = nc.dram_tensor("b1b", [128, 32], F32, kind="ExternalInput")
    w2aug = nc.dram_tensor("w2aug", [INNER + 1, DIM], BF16, kind="ExternalInput")
    onesrow = nc.dram_tensor("onesrow", [1, RB], BF16, kind="ExternalInput")
    warm = nc.dram_tensor("warm", [1, 128], F32)
    out_d = nc.dram_tensor("out", [RLOC, DIM], F32, kind="ExternalOutput")

    kb_bounce = [nc.dram_tensor(f"kTb{b}", [128, NDT, RB], F8)
                 for b in range(B)]
    G = [nc.dram_tensor(f"G{b}", [NCORES * 128, NDT, RB], F8,
                        addr_space="Shared") for b in range(B)]
    Gwarm = nc.dram_tensor("Gwarm", [NCORES, 128], F32, addr_space="Shared")

    groups = [list(range(NCORES))]

    with tile.TileContext(nc) as tc:
        with (
            tc.tile_pool(name="big", bufs=1) as big,
            tc.tile_pool(name="tmp", bufs=2) as tmp,
            tc.tile_pool(name="stg", bufs=2) as stg,
            tc.tile_pool(name="rsp", bufs=2) as rsp,
            tc.tile_pool(name="obp", bufs=4) as obp,
            tc.tile_pool(name="ps", bufs=6, space="PSUM") as pps,
            tc.tile_pool(name="po", bufs=2, space="PSUM") as ppo,
        ):
            # ---- ncfw warmup: near-dep-free tiny AllGather first on the
            # cc stream, so the ~55us ncfw startup barrier runs at the
            # front of the NEFF instead of gating the first real gather
            warm_sb = big.tile([1, 128], F32, tag="warm")
            nc.vector.memset(warm_sb[:], 0.0)
            nc.sync.dma_start(warm[:], warm_sb[:])
            nc.gpsimd.collective_compute(
                "AllGather", mybir.AluOpType.bypass, replica_groups=groups,
                ins=[warm.ap().opt()], outs=[Gwarm.ap().opt()])

            # ---- inputs -> SBUF as DoubleRow pair tiles, spread on queues --
            # (dp p k2 r) views land each pair tile in ONE dma each
            h_r = hT.rearrange("(dp k2 p) r -> dp p k2 r", dp=NDP, k2=2, p=128)
            wk_r = wkT.rearrange("(dp k2 p) r -> dp p k2 r",
                                 dp=NDP, k2=2, p=128)
            wq_r = wqT.rearrange("(dp k2 p) r -> dp p k2 r",
                                 dp=NDP, k2=2, p=128)
            h_t, wk_t, wq_t = [], [], []
            for dp in range(NDP):
                th = big.tile([128, 2, RLOC], F8, tag=f"h{dp}", name=f"h{dp}")
                nc.sync.dma_start(th[:], h_r[dp])
                h_t.append(th)
                tw = big.tile([128, 2, DIM], F8, tag=f"wk{dp}", name=f"wk{dp}")
                nc.scalar.dma_start(tw[:], wk_r[dp])
                wk_t.append(tw)
            cos_t, sin_t = [], []
            for ci in range(NDT // 2):
                tc_ = big.tile([128, RLOC], BF16, tag=f"cos{ci}",
                               name=f"cos{ci}")
                nc.scalar.dma_start(tc_[:], cosh[128 * ci:128 * (ci + 1), :])
                cos_t.append(tc_)
                ts_ = big.tile([128, RLOC], BF16, tag=f"sin{ci}",
                               name=f"sin{ci}")
                nc.scalar.dma_start(ts_[:], sinh[128 * ci:128 * (ci + 1), :])
                sin_t.append(ts_)
            for dp in range(NDP):
                # wq reuses wk's slots (k projection is done by then)
                tw = big.tile([128, 2, DIM], F8, tag=f"wk{dp}", name=f"wq{dp}")
                nc.sync.dma_start(tw[:], wq_r[dp])
                wq_t.append(tw)
            mask_sb = [big.tile([128, 512], F32, tag=f"mask{h}",
                                name=f"mask_sb{h}") for h in range(2)]
            nc.scalar.dma_start(mask_sb[0][:], mask0[:])
            nc.scalar.dma_start(mask_sb[1][:], mask1[:])
            w1b_sb = big.tile([128, 32], F32, tag="w1b")
            nc.scalar.dma_start(w1b_sb[:], w1b_d[:])
            b1b_sb = big.tile([128, 32], F32, tag="b1b")
            nc.scalar.dma_start(b1b_sb[:], b1b_d[:])
            w2_sb = big.tile([INNER + 1, DIM], BF16, tag="w2")
            nc.scalar.dma_start(w2_sb[:], w2aug[:])
            z_aug = big.tile([INNER + 1, RB], BF16, tag="zaug")
            nc.scalar.dma_start(z_aug[INNER:INNER + 1, :], onesrow[:])

            def project_half(w_t, proj, rt, bounce=False):
                """proj[:, :, 512rt:512rt+512] = fp8(rope(W @ h^T)).
                DoubleRow fp8 matmuls -> psum f32 -> bf16 staging (gpsimd)
                -> rope on vector -> fp8 slots (dt, dt+4); do-order
                interleaves the (dt, dt+4) halves so RoPE pairs complete
                (and optionally bounce to DRAM) right behind PE."""
                cols = slice(512 * rt, 512 * (rt + 1))
                pbf = stg.tile([128, NDT, 512], BF16, tag="pbf",
                               name=f"pbf{rt}")

                def rope_pair(dt):
                    cm = cos_t[dt][:, cols]
                    sm = sin_t[dt][:, cols]
                    lo = pbf[:, dt, :]
                    hi = pbf[:, dt + NDT // 2, :]
                    ta = tmp.tile([128, 512], BF16, tag="ta", name="ta")
                    tb = tmp.tile([128, 512], BF16, tag="tb", name="tb")
                    td = tmp.tile([128, 512], BF16, tag="td", name="td")
                    nc.vector.tensor_mul(ta[:], lo, cm)
                    nc.vector.tensor_mul(tb[:], lo, sm)
                    nc.vector.tensor_mul(td[:], hi, sm)
                    nc.vector.tensor_sub(proj[:, dt, cols], ta[:], td[:])
                    nc.vector.tensor_mul(ta[:], hi, cm)
                    nc.vector.tensor_add(proj[:, dt + NDT // 2, cols],
                                         ta[:], tb[:])
                    if bounce:
                        # both rope slots of the pair in one strided dma
                        eng = nc.sync if dt % 2 else nc.scalar
                        eng.dma_start(
                            kb_bounce[rt][:, dt::NDT // 2, :],
                            proj[:, dt::NDT // 2, cols])

                order = [x for pair in zip(range(NDT // 2),
                                           range(NDT // 2, NDT))
                         for x in pair]            # 0,4,1,5,2,6,3,7
                for do in order:
                    ps = pps.tile([128, 512], F32, tag="ps",
                                  name=f"psp{rt}{do}")
                    for dp in range(NDP):
                        nc.tensor.matmul(
                            ps[:], w_t[dp][:, :, 128 * do:128 * (do + 1)],
                            h_t[dp][:, :, cols],
                            start=(dp == 0), stop=(dp == NDP - 1),
                            perf_mode=DR,
                        )
                    # f32 psum -> bf16 staging for rope (reference casts
                    # q/k to bf16 here); scalar ACT keeps vector free for
                    # rope and unblocks psum banks for the next matmuls
                    nc.scalar.activation(pbf[:, do, :], ps[:],
                                         mybir.ActivationFunctionType.Copy)
                    if do >= NDT // 2:
                        rope_pair(do - NDT // 2)

            # ---- gathered-K load helper ------------------------------------
            g_r = [G[b].rearrange("(r p) t (kb jj) -> r p t kb jj",
                                  r=NCORES, p=128, kb=NSUB, jj=128)
                   for b in range(B)]
            _kteng = {0: [nc.sync, nc.scalar],
                      1: [nc.sync, nc.gpsimd]}

            def load_kt(b, hh):
                # kt layout: [128 (d in tile), t8, kb4, r4, jj128]; score
                # matmuls slice [:, 2dp:2dp+2, kb, :, :] which flattens to
                # the DoubleRow [128, 2, 512] moving shape (kb-major puts
                # the block's (r, jj) columns contiguous in SBUF).
                # Pieces are split by t-half with the kb01 halves emitted
                # first, so the first score chunks wait for only half the
                # transfer; queues round-robin over the allowed engines.
                kt = big.tile([128, NDT, NSUB, 4, 128], F8, tag=f"kt{hh}",
                              name=f"kt{b}{hh}")
                engs = _kteng[b]
                i = hh
                for r in range(4):
                    for dh in range(2):
                        eng = engs[i % len(engs)]
                        i += 1
                        eng.dma_start(
                            kt[:, 4 * dh:4 * (dh + 1), :, r, :],
                            g_r[b][4 * hh + r, :, 4 * dh:4 * (dh + 1), :, :])
                return kt

            # ---- k per batch: project+rope+bounce, then both all-gathers
            # back to back on the cc stream; q projects during the
            # collectives; kt loads are emitted last so no engine stream
            # has compute queued behind a gather-gated dma issue ---------
            k_rope = big.tile([128, NDT, RLOC], F8, tag="krope")
            project_half(wk_t, k_rope, 0, bounce=True)
            nc.gpsimd.collective_compute(
                "AllGather", mybir.AluOpType.bypass, replica_groups=groups,
                ins=[kb_bounce[0].ap().opt()], outs=[G[0].ap().opt()])
            project_half(wk_t, k_rope, 1, bounce=True)
            nc.gpsimd.collective_compute(
                "AllGather", mybir.AluOpType.bypass, replica_groups=groups,
                ins=[kb_bounce[1].ap().opt()], outs=[G[1].ap().opt()])

            # ---- q: project + rope (overlaps with the collectives) ----
            q_rope = big.tile([128, NDT, RLOC], F8, tag="qrope")
            project_half(wq_t, q_rope, 0)
            project_half(wq_t, q_rope, 1)

            kt0 = [load_kt(0, hh) for hh in range(2)]
            kt1 = [load_kt(1, hh) for hh in range(2)]

            o_sb = big.tile([128, B * NSUB], F32, tag="o")

            def scores(b, kts):
                rs_t = [rsp.tile([128, 2 * NSUB], F32, tag=f"rs{s}",
                                 name=f"rs{b}{s}") for s in range(NSUB)]
                for hh in range(2):
                    for s in range(NSUB):
                        blocks = list(range(s + 1))
                        for c0 in range(0, len(blocks), CHUNK):
                            chunk = blocks[c0:c0 + CHUNK]
                            psl = [pps.tile([128, 512], F32, tag="ps",
                                            name=f"ps{b}{hh}{s}{c0}_{i}")
                                   for i in range(len(chunk))]
                            for dp in range(NDP):
                                lhsT = q_rope[:, 2 * dp:2 * dp + 2,
                                              RB * b + 128 * s:
                                              RB * b + 128 * (s + 1)]
                                for kb, ps in zip(chunk, psl):
                                    nc.tensor.matmul(
                                        ps[:], lhsT,
                                        kts[hh][:, 2 * dp:2 * dp + 2,
                                                kb, :, :],
                                        start=(dp == 0), stop=(dp == NDP - 1),
                                        perf_mode=DR,
                                    )
                            for kb, ps in zip(chunk, psl):
                                if kb == s:
                                    nc.vector.tensor_add(ps[:], ps[:],
                                                         mask_sb[hh][:])
                                nc.scalar.activation(
                                    ps[:], ps[:],
                                    mybir.ActivationFunctionType.Exp,
                                    scale=SCALE,
                                    accum_out=rs_t[s][:, 2 * kb + hh:
                                                      2 * kb + hh + 1],
                                )
                for s in range(NSUB):
                    nc.vector.reduce_sum(
                        o_sb[:, NSUB * b + s:NSUB * b + s + 1],
                        rs_t[s][:, 0:2 * (s + 1)], axis=mybir.AxisListType.X)
                    mlp_sub(b, s)

            def mlp_sub(b, s):
                # o_sb[p, b*NSUB+s] is local row b*RB + 128s + p.
                # z[row, n] = relu(o[row]*w1[n] + b1[n]) with o as a
                # per-partition scalar, DVE-transposed into z_aug[n, row],
                # then out rows = z_aug.T @ w2aug, DMAed straight from PSUM.
                col = NSUB * b + s
                zrow = tmp.tile([128, 32], F32, tag="zr", name=f"zr{b}{s}")
                nc.vector.tensor_scalar_mul(zrow[:], w1b_sb[:],
                                            o_sb[:, col:col + 1])
                nc.vector.tensor_add(zrow[:], zrow[:], b1b_sb[:])
                zrb = tmp.tile([128, 32], BF16, tag="zrb",
                               name=f"zrb{b}{s}")
                nc.vector.tensor_scalar_max(zrb[:], zrow[:], 0.0)
                zts = tmp.tile([32, 128], BF16, tag="zts", name=f"zts{b}{s}")
                for g in range(4):
                    nc.vector.transpose(zts[0:32, 32 * g:32 * (g + 1)],
                                        zrb[32 * g:32 * (g + 1), :])
                nc.vector.tensor_copy(z_aug[0:INNER, 128 * s:128 * (s + 1)],
                                      zts[0:INNER, :])
                row0 = RB * b + 128 * s
                ob = obp.tile([128, DIM], F32, tag="ob", name=f"ob{b}{s}")
                for hh in range(2):
                    po = ppo.tile([128, 512], F32, tag="po",
                                  name=f"po{b}{s}{hh}")
                    nc.tensor.matmul(po[:],
                                     z_aug[:, 128 * s:128 * (s + 1)],
                                     w2_sb[:, 512 * hh:512 * (hh + 1)],
                                     start=True, stop=True)
                    nc.vector.tensor_copy(ob[:, 512 * hh:512 * (hh + 1)],
                                          po[:])
                eng = nc.gpsimd if s % 2 else nc.sync
                eng.dma_start(out_d[row0:row0 + 128, :], ob[:])

            scores(0, kt0)
            scores(1, kt1)

    nc.compile()
    return nc


def get_nc():
    if "nc" not in _NC_CACHE:
        _NC_CACHE["nc"] = _build_nc()
    return _NC_CACHE["nc"]


def make_in_maps(hidden_states, Wq, Wk, fc1_w, fc1_b, fc2_w, fc2_b):
    hidden_states = np.asarray(hidden_states, dtype=np.float32)
    Wq = np.asarray(Wq, dtype=np.float32)
    Wk = np.asarray(Wk, dtype=np.float32)
    fc1_w = np.asarray(fc1_w, dtype=np.float32)
    fc1_b = np.asarray(fc1_b, dtype=np.float32)
    fc2_w = np.asarray(fc2_w, dtype=np.float32)
    fc2_b = np.asarray(fc2_b, dtype=np.float32)

    wqT = np.ascontiguousarray(Wq.T).astype(float8_e4m3)
    wkT = np.ascontiguousarray(Wk.T).astype(float8_e4m3)
    w1b = np.zeros((128, 32), dtype=np.float32)
    w1b[:, 0:INNER] = fc1_w.reshape(1, INNER)
    b1b = np.zeros((128, 32), dtype=np.float32)
    b1b[:, 0:INNER] = fc1_b.reshape(1, INNER)
    w2aug = np.concatenate([fc2_w.T, fc2_b[None, :]], axis=0).astype(bfloat16)

    inv_freq = ROPE_BASE ** (-np.arange(0, DIM, 2, dtype=np.float32) / DIM)

    in_maps = []
    for c in range(NCORES):
        rows = np.arange(RB) * NCORES + c            # global rows, per batch
        hT = np.concatenate(
            [hidden_states[b, rows, :].T for b in range(B)],
            axis=1).astype(float8_e4m3)              # [DIM, RLOC]
        ang = rows[:, None].astype(np.float32) * inv_freq[None, :]  # [RB,512]
        ch = np.cos(ang).T.astype(np.float32)        # [512, RB]
        sh = np.sin(ang).T.astype(np.float32)
        cosh = np.concatenate([ch, ch], axis=1).astype(bfloat16)
        sinh = np.concatenate([sh, sh], axis=1).astype(bfloat16)
        # mask_h[p, (jc-4h)*128+t]: allow k col (rank jc, t) for q row p iff
        # 8t + jc <= 8p + c  (boundary subtile; same for every s and batch)
        p = np.arange(128)[:, None, None]
        t = np.arange(128)[None, None, :]
        masks = []
        for h in range(2):
            jc = (np.arange(4) + 4 * h)[None, :, None]
            allow = (NCORES * t + jc) <= (NCORES * p + c)
            masks.append(np.where(allow, 0.0, MASK_NEG)
                         .astype(np.float32).reshape(128, 512))
        in_maps.append({
            "hT": np.ascontiguousarray(hT),
            "wqT": wqT, "wkT": wkT,
            "cosh": np.ascontiguousarray(cosh),
            "sinh": np.ascontiguousarray(sinh),
            "mask0": masks[0], "mask1": masks[1],
            "w1b": w1b, "b1b": b1b, "w2aug": w2aug,
            "onesrow": np.ones((1, RB), dtype=bfloat16),
        })
    return in_maps


def assemble_output(results):
    out = np.empty((B, L, DIM), dtype=np.float32)
    for c in range(NCORES):
        for b in range(B):
            out[b, c::NCORES, :] = results[c]["out"][RB * b:RB * (b + 1)]
    return out


def run(trace=False, **inputs):
    nc = get_nc()
    in_maps = make_in_maps(**inputs)
    res = run_bass_kernel_spmd(nc, in_maps, core_ids=list(range(NCORES)),
                               trace=trace)
    return assemble_output(res.results), res


def kernel(**inputs) -> np.ndarray:
    out, _ = run(trace=False, **inputs)
    return out
